# revision 25
# baseline (speedup 1.0000x reference)
"""Trainium2 Bass kernel for nn_AgentModule (multi-agent GRU game module).

Strategy v2 (collapsed utterance pathway):
 - mem_utterance is zero-filled and utt_x[i,j]=utterances[j] is independent of
   the receiving agent i, so the [A,A,H] utterance GRU state stays i-uniform
   for all T steps: the whole utterance pathway collapses exactly to [A,H].
   Each core computes the collapsed path (replicated, tiny) and scales the
   goal-prediction cost by AC agents via the Square activation scale.
 - Physical pathway shards agents across 8 cores (AC=16, NP=AC*E=4096 rows),
   feature-major layout h^T [H, NP] in SBUF bf16.
 - Gate evacuation touches each PSUM word once: rz gates leave PSUM through
   ACT tanh(0.5x+b/2) (sigmoid kept in tanh form); the static rank-5 input
   term is accumulated on the PE as an extra K-chunk; the n-gate static term
   giPn stays SBUF-resident and is folded in with one gpsimd add.
 - ELU decomposition elu(x)+1 = min(exp(x),1) + relu(x): per-agent pooled
   sums come from fused STT accum_out, no full reduce instructions.
 - One ACT table set (exp_and_others: tanh/exp/relu/square) for the whole
   kernel; engine balance PE/ACT/DVE/GPSIMD tuned per cost model.
"""

import sys

for _p in ("/opt/trn_rl_repo", "/opt/pypackages"):
    if _p not in sys.path:
        sys.path.append(_p)

import numpy as np
import ml_dtypes

import concourse.bass as bass
import concourse.bacc as bacc
import concourse.mybir as mybir
import concourse.tile as tile
from concourse.bass_utils import run_bass_kernel_spmd

F32 = mybir.dt.float32
BF16 = mybir.dt.bfloat16
AF = mybir.ActivationFunctionType
OP = mybir.AluOpType
AX = mybir.AxisListType

NCORES = 8
A = 128
L = 128
E = A + L          # 256
T = 8
H = 256
V = 32
GS = 5
GI = 5
OD = 2
PD = 3
MD = 2
STEP = 0.1
EPS = 1e-20
AC = A // NCORES   # 16 agents per core
NP = AC * E        # 4096 physical rows per core
CB = 1024          # column-block width for the P path
NCB = NP // CB     # 4

# bias matrix columns
BRZ_P = 0     # 4 cols: (p_bih+p_bhh)[:512]/2
BHN_P = 4     # 2 cols: p_bhh[512:768]
BFC_P = 6     # 2 cols: p_bfc
BGP2 = 8      # 1 col (5 rows): 4*(gp_b2 - gp_W2.sum(1))
BONEC = 9     # ones column
BNRZ_G = 10   # 4 cols: g_bhh[:512]/2
BHN_G = 14    # 2 cols: g_bhh[512:768]
BFC_G = 16    # 2 cols: g_bfc
BNRZ_A = 18   # 4 cols: (a_bih_adj[:512]+a_bhh[:512])/2
BIN_A = 22    # 2 cols: a_bih_adj[512:768]
BHN_A = 24    # 2 cols: a_bhh[512:768]
BFC_A = 26    # 2 cols: a_bfc
BM1 = 28      # 2 cols: m_b1 - m_W1.sum(1)
BM2 = 30     # 1 col (2 rows): m_b2 - m_W2.sum(1)
BUC1 = 31     # 2 cols: uc_b1 - uc_W1.sum(1)
BUC2R = 33    # row0 cols 33:65: uc_b2 - uc_W2.sum(1)
BONER = 65    # row0 cols 65:81: ones(16)
NBIAS = 82

# urows segments (bf16 bias rows for the U path PE bias-passes)
UR_RZ = 0       # 512: (u_bih+u_bhh)[:512]
UR_BHN = 512    # 256: u_bhh[512:768]
UR_FC = 768     # 256: u_bfc
UR_GP1 = 1024   # 256: gp_b1 - gp_W1.sum(1)
UR_ONE = 1280   # 128: ones
URN = 1408

_DRAM_SPECS = [
    # P path
    ("hP0", (H, NP), BF16),
    ("giPn", (H, NP), BF16),
    ("xpT", (OD + PD, NP), BF16),
    ("whhP", (H, 3 * H), BF16),
    ("wihPrz", (OD + PD, 2 * H), BF16),
    ("wfcP", (H, H), BF16),
    ("bhnP", (1, H), BF16),
    ("onesr", (1, 1024), BF16),
    # U path (collapsed, replicated; packed [128, 2*128] layout)
    ("hU0", (128, 2 * A), BF16),
    ("giUn", (128, 2 * A), BF16),
    ("xuT", (V, A), BF16),
    ("whhU", (H, 3 * H), BF16),
    ("wihUrz", (V, 2 * H), BF16),
    ("wfcU", (H, H), BF16),
    ("wgp1", (H, H), BF16),
    ("wgp2", (H, GS), BF16),
    ("urows", (1, URN), BF16),
    # action module
    ("memaT", (H, AC), BF16),
    ("ggiT", (3 * H, AC), F32),
    ("gumb", (AC, T * V), F32),
    ("whhG", (H, 3 * H), BF16),
    ("wfcG", (H, H), BF16),
    ("wihA", (3 * H, 3 * H), BF16),
    ("whhA", (H, 3 * H), BF16),
    ("wfcA", (H, H), BF16),
    ("wm1", (H, H), BF16),
    ("wm2", (H, MD), BF16),
    ("wuc1", (H, H), BF16),
    ("wuc2", (H, V), BF16),
    ("biases", (128, NBIAS), F32),
]


def _emit(tc, D, cost_out):
    nc = tc.nc
    import contextlib

    stack = contextlib.ExitStack()
    pers = stack.enter_context(tc.tile_pool(name="pers", bufs=1))

    def persist(name, shape, dtype=F32):
        return pers.tile(list(shape), dtype, tag=name, name=name)

    def load2(name, rows, cols, dtype=F32, ptile=128):
        nt = (rows + ptile - 1) // ptile
        out = []
        for k in range(nt):
            p = min(ptile, rows - k * ptile)
            tl = persist(f"{name}_{k}", (p, cols), dtype)
            nc.sync.dma_start(tl[:], D[name][k * ptile : k * ptile + p, :])
            out.append(tl)
        return out

    # ---------------- persistent state + weights ----------------
    hP = load2("hP0", H, NP, dtype=BF16)          # 2 x [128, 4096]
    giPn = load2("giPn", H, NP, dtype=BF16)       # 2 x [128, 4096]
    xpT = load2("xpT", OD + PD, NP, dtype=BF16)[0]
    whhP = load2("whhP", H, 3 * H, dtype=BF16)
    wihPrz = load2("wihPrz", OD + PD, 2 * H, dtype=BF16)[0]
    wfcP = load2("wfcP", H, H, dtype=BF16)
    bhnP = load2("bhnP", 1, H, dtype=BF16)[0]
    onesr = load2("onesr", 1, 1024, dtype=BF16)[0]

    hU = load2("hU0", 128, 2 * A, dtype=BF16)[0]  # packed [128, 256]
    giUn = load2("giUn", 128, 2 * A, dtype=BF16)[0]
    xuT = load2("xuT", V, A, dtype=BF16)[0]
    whhU = load2("whhU", H, 3 * H, dtype=BF16)
    wihUrz = load2("wihUrz", V, 2 * H, dtype=BF16)[0]
    wfcU = load2("wfcU", H, H, dtype=BF16)
    wgp1 = load2("wgp1", H, H, dtype=BF16)
    wgp2 = load2("wgp2", H, GS, dtype=BF16)
    urows = load2("urows", 1, URN, dtype=BF16)[0]

    hA = load2("memaT", H, AC, dtype=BF16)
    ggiT = load2("ggiT", 3 * H, AC)
    gumb = load2("gumb", AC, T * V)[0]
    whhG = load2("whhG", H, 3 * H, dtype=BF16)
    wfcG = load2("wfcG", H, H, dtype=BF16)
    wihA = load2("wihA", 3 * H, 3 * H, dtype=BF16)
    whhA = load2("whhA", H, 3 * H, dtype=BF16)
    wfcA = load2("wfcA", H, H, dtype=BF16)
    wm1 = load2("wm1", H, H, dtype=BF16)
    wm2 = load2("wm2", H, MD, dtype=BF16)
    wuc1 = load2("wuc1", H, H, dtype=BF16)
    wuc2 = load2("wuc2", H, V, dtype=BF16)
    bia = load2("biases", 128, NBIAS)[0]

    def bvec(idx, p=128):
        return bia[:p, idx : idx + 1]

    cost_buf = persist("cost_buf", (128, 8 * T))
    nc.vector.memset(cost_buf[:], 0.0)
    ones16 = persist("ones16", (128, AC), BF16)
    nc.vector.memset(ones16[:], 1.0)

    # ---------------- pools ----------------
    psG = stack.enter_context(tc.tile_pool(name="psG", bufs=3, space="PSUM"))
    psS = stack.enter_context(tc.tile_pool(name="psS", bufs=2, space="PSUM"))
    psF = psG
    p_tr = stack.enter_context(tc.tile_pool(name="p_tr", bufs=4))
    p_tz = stack.enter_context(tc.tile_pool(name="p_tz", bufs=8))
    p_rf = stack.enter_context(tc.tile_pool(name="p_rf", bufs=8))
    p_t1 = stack.enter_context(tc.tile_pool(name="p_t1", bufs=3))
    p_t2 = stack.enter_context(tc.tile_pool(name="p_t2", bufs=3))
    p_n = stack.enter_context(tc.tile_pool(name="p_n", bufs=8))
    p_d = stack.enter_context(tc.tile_pool(name="p_d", bufs=3))
    p_e = stack.enter_context(tc.tile_pool(name="p_e", bufs=3))
    p_e1 = stack.enter_context(tc.tile_pool(name="p_e1", bufs=3))
    p_r1 = stack.enter_context(tc.tile_pool(name="p_r1", bufs=3))
    p_scr = stack.enter_context(tc.tile_pool(name="p_scr", bufs=4))
    p_u = stack.enter_context(tc.tile_pool(name="p_u", bufs=2))
    p_ubar = stack.enter_context(tc.tile_pool(name="p_ubar", bufs=2))
    p_sums = stack.enter_context(tc.tile_pool(name="p_sums", bufs=2))
    p_small = stack.enter_context(tc.tile_pool(name="p_small", bufs=2))

    def mm(out, lhsT, rhs, start, stop):
        nc.tensor.matmul(out, lhsT, rhs, start=start, stop=stop)

    # ================= U path (collapsed utterance + goal pred) =============
    def u_path(t):
        # gates: rz packed [128, 512] (4 Mtiles of 128 agents)
        psu = psS.tile([128, 4 * A], F32, tag="sm", name=f"psu_{t}")
        for m in range(4):
            sl = psu[:, m * A : (m + 1) * A]
            mm(sl, whhU[0][:, m * 128 : m * 128 + 128], hU[:, 0:A], True, False)
            mm(sl, whhU[1][:, m * 128 : m * 128 + 128], hU[:, A : 2 * A], False, False)
            mm(sl, wihUrz[:, m * 128 : m * 128 + 128], xuT[:], False, False)
            mm(sl, urows[:, UR_RZ + m * 128 : UR_RZ + m * 128 + 128],
               urows[:, UR_ONE : UR_ONE + A], False, True)
        trzu = p_u.tile([128, 4 * A], BF16, tag="trzu", name=f"trzu_{t}")
        nc.scalar.activation(trzu[:], psu[:], AF.Tanh, scale=0.5)
        rfu = p_u.tile([128, 2 * A], BF16, tag="rfu", name=f"rfu_{t}")
        nc.vector.tensor_scalar(rfu[:], trzu[:, 0 : 2 * A], 0.5, 0.5, OP.mult, OP.add)
        # n gates packed [128, 256] (+ bhn bias-pass, gi stays outside)
        psn = psS.tile([128, 2 * A], F32, tag="sm", name=f"psn_{t}")
        for k in range(2):
            m = 4 + k
            sl = psn[:, k * A : (k + 1) * A]
            mm(sl, whhU[0][:, m * 128 : m * 128 + 128], hU[:, 0:A], True, False)
            mm(sl, whhU[1][:, m * 128 : m * 128 + 128], hU[:, A : 2 * A], False, False)
            mm(sl, urows[:, UR_BHN + k * 128 : UR_BHN + k * 128 + 128],
               urows[:, UR_ONE : UR_ONE + A], False, True)
        t1u = p_u.tile([128, 2 * A], BF16, tag="t1u", name=f"t1u_{t}")
        nc.vector.tensor_mul(t1u[:], psn[:], rfu[:])
        t2u = p_u.tile([128, 2 * A], BF16, tag="t2u", name=f"t2u_{t}")
        nc.vector.tensor_add(t2u[:], t1u[:], giUn[:])
        nu = p_u.tile([128, 2 * A], BF16, tag="nu", name=f"nu_{t}")
        nc.scalar.activation(nu[:], t2u[:], AF.Tanh)
        du = p_u.tile([128, 2 * A], BF16, tag="du", name=f"du_{t}")
        nc.vector.tensor_sub(du[:], hU[:], nu[:])
        eu = p_u.tile([128, 2 * A], BF16, tag="eu", name=f"eu_{t}")
        nc.vector.scalar_tensor_tensor(eu[:], trzu[:, 2 * A : 4 * A], 1.0, du[:],
                                       OP.add, OP.mult)
        nc.vector.scalar_tensor_tensor(hU[:], eu[:], 0.5, nu[:], OP.mult, OP.add)

    def u_fc(t):
        # fc -> m_u (= utt_proc+1) + ubar accum
        psf = psS.tile([128, 2 * A], F32, tag="sm", name=f"psf_{t}")
        for mf in range(2):
            sl = psf[:, mf * A : (mf + 1) * A]
            mm(sl, wfcU[0][:, mf * 128 : mf * 128 + 128], hU[:, 0:A], True, False)
            mm(sl, wfcU[1][:, mf * 128 : mf * 128 + 128], hU[:, A : 2 * A], False, False)
            mm(sl, urows[:, UR_FC + mf * 128 : UR_FC + mf * 128 + 128],
               urows[:, UR_ONE : UR_ONE + A], False, True)
        e1u = p_u.tile([128, 2 * A], BF16, tag="e1u", name=f"e1u_{t}")
        nc.scalar.activation(e1u[:], psf[:], AF.Exp)
        r1u = p_u.tile([128, 2 * A], BF16, tag="r1u", name=f"r1u_{t}")
        nc.vector.tensor_scalar(r1u[:], psf[:], 0.0, 1.0, OP.max, OP.add)
        mu = p_u.tile([128, 2 * A], BF16, tag="mu", name=f"mu_{t}")
        ubar = []
        for c in range(2):
            ub_c = p_ubar.tile([128, 1], F32, tag=f"ubar{c}", name=f"ubar{c}_{t}")
            sl = slice(c * A, (c + 1) * A)
            nc.vector.scalar_tensor_tensor(
                mu[:, sl], e1u[:, sl], 1.0, r1u[:, sl], OP.mult, OP.min,
                accum_out=ub_c[:],
            )
            ubar.append(ub_c)
        # goal prediction: gp1 (elu, shifted) -> gp2 -> squared cost
        psg = psS.tile([128, 2 * A], F32, tag="sm", name=f"psg_{t}")
        for mf in range(2):
            sl = psg[:, mf * A : (mf + 1) * A]
            mm(sl, wgp1[0][:, mf * 128 : mf * 128 + 128], mu[:, 0:A], True, False)
            mm(sl, wgp1[1][:, mf * 128 : mf * 128 + 128], mu[:, A : 2 * A], False, False)
            mm(sl, urows[:, UR_GP1 + mf * 128 : UR_GP1 + mf * 128 + 128],
               urows[:, UR_ONE : UR_ONE + A], False, True)
        e1g = p_u.tile([128, 2 * A], BF16, tag="e1g", name=f"e1g_{t}")
        nc.scalar.activation(e1g[:], psg[:], AF.Exp)
        r1g = p_u.tile([128, 2 * A], BF16, tag="r1g", name=f"r1g_{t}")
        nc.vector.tensor_scalar(r1g[:], psg[:], 0.0, 1.0, OP.max, OP.add)
        y1 = p_u.tile([128, 2 * A], BF16, tag="y1", name=f"y1_{t}")
        nc.vector.scalar_tensor_tensor(y1[:], e1g[:], 1.0, r1g[:], OP.mult, OP.min)
        ps5 = psS.tile([GS, A], F32, tag="sm", name=f"ps5_{t}")
        mm(ps5[:], wgp2[0][:, :], y1[:, 0:A], True, False)
        mm(ps5[:], wgp2[1][:, :], y1[:, A : 2 * A], False, True)
        sq = p_u.tile([GS, A], BF16, tag="sq", name=f"sq_{t}")
        # cost += AC * sum(goal_pred^2): (4x+4b')^2 = 16 (x+b')^2
        nc.scalar.activation(
            sq[:], ps5[:], AF.Square, bias=bvec(BGP2, GS), scale=4.0,
            accum_out=cost_buf[:GS, 8 * t : 8 * t + 1],
        )
        return ubar

    # ================= P path (physical processor) ===========================
    def p_path(t, sums_p):
        trz = {}
        rfull = {}
        nts = {}
        for m in (2, 3, 0, 1, 4, 5):
            units = []
            for cb in range(NCB):
                u = psG.tile([128, CB], F32, tag="gps", name=f"gps_{t}_{m}_{cb}")
                units.append(u)
            is_n = m >= 4
            nk = 3 if not is_n else 2
            for k in range(2):
                lhsT = whhP[k][:, m * 128 : m * 128 + 128]
                for cb in range(NCB):
                    for s in range(2):
                        mm(units[cb][:, s * 512 : (s + 1) * 512], lhsT,
                           hP[k][:, cb * CB + s * 512 : cb * CB + s * 512 + 512],
                           k == 0, k == nk - 1)
            if not is_n:
                lhsT = wihPrz[:, m * 128 : m * 128 + 128]
                for cb in range(NCB):
                    for s in range(2):
                        mm(units[cb][:, s * 512 : (s + 1) * 512], lhsT,
                           xpT[:, cb * CB + s * 512 : cb * CB + s * 512 + 512],
                           False, True)
            if m < 2:           # r gates: tanh form then sigmoid affine
                rfull[m] = []
                for cb in range(NCB):
                    tr = p_tr.tile([128, CB], BF16, tag="tr")
                    nc.scalar.activation(tr[:], units[cb][:], AF.Tanh,
                                         bias=bvec(BRZ_P + m), scale=0.5)
                    rf = p_rf.tile([128, CB], BF16, tag="rf",
                                   name=f"rf_{t}_{m}_{cb}")
                    nc.vector.tensor_scalar(rf[:], tr[:], 0.5, 0.5, OP.mult, OP.add)
                    rfull[m].append(rf)
            elif m >= 4:        # n gates
                k2 = m - 4
                nts[k2] = []
                for cb in range(NCB):
                    t1 = p_t1.tile([128, CB], BF16, tag="t1")
                    nc.vector.scalar_tensor_tensor(
                        t1[:], units[cb][:], bvec(BHN_P + k2), rfull[k2][cb][:],
                        OP.add, OP.mult)
                    t2 = p_t2.tile([128, CB], BF16, tag="t2")
                    nc.gpsimd.tensor_add(
                        t2[:], t1[:], giPn[k2][:, cb * CB : (cb + 1) * CB])
                    n_ = p_n.tile([128, CB], BF16, tag="n",
                                  name=f"n_{t}_{k2}_{cb}")
                    nc.scalar.activation(n_[:], t2[:], AF.Tanh)
                    nts[k2].append(n_)
            else:               # z gates -> sigmoid form
                trz[m - 2] = []
                for cb in range(NCB):
                    tzt = p_tr.tile([128, CB], BF16, tag="tr")
                    nc.scalar.activation(tzt[:], units[cb][:], AF.Tanh,
                                         bias=bvec(BRZ_P + m), scale=0.5)
                    tz = p_tz.tile([128, CB], BF16, tag="tz",
                                   name=f"tz_{t}_{m}_{cb}")
                    nc.vector.tensor_scalar(tz[:], tzt[:], 0.5, 0.5,
                                            OP.mult, OP.add)
                    trz[m - 2].append(tz)
        # blend: h' = n + 0.5(1+tz)(h-n)
        for k in range(2):
            for cb in range(NCB):
                hk = hP[k][:, cb * CB : (cb + 1) * CB]
                d = p_d.tile([128, CB], BF16, tag="d")
                nc.gpsimd.tensor_sub(d[:], hk, nts[k][cb][:])
                e_ = p_e.tile([128, CB], BF16, tag="e")
                nc.vector.tensor_mul(e_[:], trz[k][cb][:], d[:])
                nc.vector.tensor_add(hk, e_[:], nts[k][cb][:])
        # fc -> per-agent pooled sums (elu+1 = min(exp,1) + relu)
        for mf in range(2):
            funits = []
            for cb in range(NCB):
                u = psG.tile([128, CB], F32, tag="gps",
                             name=f"fps_{t}_{mf}_{cb}")
                funits.append(u)
            for k in range(2):
                lhsT = wfcP[k][:, mf * 128 : mf * 128 + 128]
                for cb in range(NCB):
                    for s in range(2):
                        mm(funits[cb][:, s * 512 : (s + 1) * 512], lhsT,
                           hP[k][:, cb * CB + s * 512 : cb * CB + s * 512 + 512],
                           k == 0, k == 1)
            for cb in range(NCB):
                e1 = p_e1.tile([128, CB], BF16, tag="e1")
                nc.scalar.activation(e1[:], funits[cb][:], AF.Exp,
                                     bias=bvec(BFC_P + mf))
                r1 = p_r1.tile([128, CB], BF16, tag="r1")
                nc.scalar.activation(r1[:], funits[cb][:], AF.Relu,
                                     bias=bvec(BFC_P + mf))
                for a in range(CB // E):
                    sl = slice(a * E, (a + 1) * E)
                    scr = p_scr.tile([128, E], BF16, tag="scr")
                    ai = cb * (CB // E) + a
                    nc.vector.scalar_tensor_tensor(
                        scr[:], e1[:, sl], 1.0, r1[:, sl], OP.min, OP.add,
                        accum_out=sums_p[mf][:, ai : ai + 1])

    # ================= action module =========================================
    def small_gru_gates(whh, hT, m, extra_k=None):
        ps = psS.tile([128, AC], F32, tag="sm")
        first = True
        if extra_k is not None:
            for ki, rhs in enumerate(extra_k):
                mm(ps[:], wihA[ki][:, m * 128 : m * 128 + 128], rhs[:], first, False)
                first = False
        mm(ps[:], whh[0][:, m * 128 : m * 128 + 128], hT[0][:], first, False)
        mm(ps[:], whh[1][:, m * 128 : m * 128 + 128], hT[1][:], False, True)
        return ps

    def small_fc(wfc, rhs, tag="sm"):
        out = []
        for mf in range(2):
            ps = psS.tile([128, AC], F32, tag=tag)
            mm(ps[:], wfc[0][:, mf * 128 : mf * 128 + 128], rhs[0][:], True, False)
            mm(ps[:], wfc[1][:, mf * 128 : mf * 128 + 128], rhs[1][:], False, True)
            out.append(ps)
        return out

    def exp_sigmoid(in_ap, hbidx, name=None):
        th = p_small.tile([128, AC], F32, tag="es", name=name)
        nc.scalar.activation(th[:], in_ap, AF.Tanh, bias=bvec(hbidx), scale=0.5)
        s_ = p_small.tile([128, AC], BF16, tag="es3", name=(name or "") + "s")
        nc.vector.tensor_scalar(s_[:], th[:], 0.5, 0.5, OP.mult, OP.add)
        return s_

    def elu_shift_small(ps, bidx, tag, p=128):
        e1 = p_small.tile([p, AC], BF16, tag=tag + "e")
        nc.scalar.activation(e1[:], ps[:], AF.Exp, bias=bvec(bidx, p))
        r1 = p_small.tile([p, AC], BF16, tag=tag + "r")
        nc.scalar.activation(r1[:], ps[:], AF.Relu, bias=bvec(bidx, p))
        m_ = p_small.tile([p, AC], BF16, tag=tag + "m")
        nc.vector.scalar_tensor_tensor(m_[:], r1[:], 1.0, e1[:], OP.add, OP.min)
        return m_

    def small_elu(ps_pair, bidx, tag):
        return [elu_shift_small(ps_pair[mf], bidx + mf, f"{tag}{mf}")
                for mf in range(2)]

    def action_pre(t):
        # goal processor GRU (state not persisted)
        grz = []
        for m in range(4):
            ps = small_gru_gates(whhG, hA, m)
            tt = p_small.tile([128, AC], F32, tag="gt")
            nc.vector.tensor_add(tt[:], ps[:], ggiT[m][:])
            grz.append(exp_sigmoid(tt[:], BNRZ_G + m, name=f"grz{m}_{t}"))
        gn = []
        for k in range(2):
            m = 4 + k
            ps = small_gru_gates(whhG, hA, m)
            t1 = p_small.tile([128, AC], BF16, tag="gt")
            nc.vector.scalar_tensor_tensor(
                t1[:], ps[:], bvec(BHN_G + k), grz[k][:], OP.add, OP.mult)
            t2 = p_small.tile([128, AC], BF16, tag="gt2")
            nc.vector.tensor_add(t2[:], t1[:], ggiT[m][:])
            n_ = p_small.tile([128, AC], BF16, tag="gn")
            nc.scalar.activation(n_[:], t2[:], AF.Tanh)
            gn.append(n_)
        g2 = []
        for k in range(2):
            d = p_small.tile([128, AC], BF16, tag="gd")
            nc.vector.tensor_sub(d[:], hA[k][:], gn[k][:])
            e_ = p_small.tile([128, AC], BF16, tag="ge")
            nc.vector.tensor_mul(e_[:], grz[2 + k][:], d[:])
            g2k = p_small.tile([128, AC], BF16, tag="g2")
            nc.vector.tensor_add(g2k[:], gn[k][:], e_[:])
            g2.append(g2k)
        mg = small_elu(small_fc(wfcG, g2), BFC_G, "mg")
        return mg

    def action_post(t, sums_p, mg, ubar):
        # broadcast utterance feature sums to AC columns
        ub = []
        for c in range(2):
            u = p_small.tile([128, AC], BF16, tag=f"ub{c}", name=f"ub{c}_{t}")
            nc.vector.tensor_scalar_mul(u[:], ones16[:], ubar[c][:, 0:1])
            ub.append(u)
        xch = []
        for xi, src_t in enumerate(
            [sums_p[0], sums_p[1], ub[0], ub[1], mg[0], mg[1]]
        ):
            xb = p_small.tile([128, AC], BF16, tag=f"xb{xi}", name=f"xb{xi}_{t}")
            nc.vector.tensor_copy(xb[:], src_t[:])
            xch.append(xb)
        arz = []
        for m in range(4):
            ps = small_gru_gates(whhA, hA, m, extra_k=xch)
            arz.append(exp_sigmoid(ps[:], BNRZ_A + m, name=f"arz{m}_{t}"))
        an = []
        for k in range(2):
            m = 4 + k
            psg = psS.tile([128, AC], F32, tag="sm")
            for ki, rhs in enumerate(xch):
                mm(psg[:], wihA[ki][:, m * 128 : m * 128 + 128], rhs[:],
                   ki == 0, ki == 5)
            psh = psS.tile([128, AC], F32, tag="sm")
            mm(psh[:], whhA[0][:, m * 128 : m * 128 + 128], hA[0][:], True, False)
            mm(psh[:], whhA[1][:, m * 128 : m * 128 + 128], hA[1][:], False, True)
            t1 = p_small.tile([128, AC], BF16, tag="at1")
            nc.vector.scalar_tensor_tensor(
                t1[:], psh[:], bvec(BHN_A + k), arz[k][:], OP.add, OP.mult)
            t2 = p_small.tile([128, AC], F32, tag="at2")
            nc.vector.scalar_tensor_tensor(
                t2[:], psg[:], bvec(BIN_A + k), t1[:], OP.add, OP.add)
            n_ = p_small.tile([128, AC], BF16, tag="an")
            nc.scalar.activation(n_[:], t2[:], AF.Tanh)
            an.append(n_)
        for k in range(2):
            d = p_small.tile([128, AC], BF16, tag="ad")
            nc.vector.tensor_sub(d[:], hA[k][:], an[k][:])
            e_ = p_small.tile([128, AC], BF16, tag="ae")
            nc.vector.tensor_mul(e_[:], arz[2 + k][:], d[:])
            nc.vector.tensor_add(hA[k][:], an[k][:], e_[:])
        mp = small_elu(small_fc(wfcA, hA), BFC_A, "mp")

        # movement chooser -> cost
        ym = small_elu(small_fc(wm1, mp), BM1, "ym")
        psmv = psS.tile([MD, AC], F32, tag="sm")
        mm(psmv[:], wm2[0][:, :], ym[0][:], True, False)
        mm(psmv[:], wm2[1][:, :], ym[1][:], False, True)
        mvt = p_small.tile([MD, AC], F32, tag="mvt")
        nc.scalar.activation(mvt[:], psmv[:], AF.Tanh, bias=bvec(BM2, MD))
        fmv = p_small.tile([MD, AC], F32, tag="fmv")
        nc.vector.tensor_scalar(fmv[:], mvt[:], 2.0 * STEP, -STEP, OP.mult, OP.add)
        scr = p_small.tile([MD, AC], F32, tag="mscr")
        nc.scalar.activation(
            scr[:], fmv[:], AF.Square,
            accum_out=cost_buf[:MD, 8 * t + 5 : 8 * t + 6])

        # utterance chooser -> gumbel softmax -> cost
        yu = small_elu(small_fc(wuc1, mp), BUC1, "yu")
        pslg = psS.tile([AC, V], F32, tag="sm")
        mm(pslg[:], yu[0][:], wuc2[0][:, :], True, False)
        mm(pslg[:], yu[1][:], wuc2[1][:, :], False, False)
        mm(pslg[:], bia[:1, BONER : BONER + AC], bia[:1, BUC2R : BUC2R + V],
           False, True)
        yg = p_small.tile([AC, V], F32, tag="yg")
        nc.vector.tensor_add(yg[:], pslg[:], gumb[:, t * V : (t + 1) * V])
        nmx = p_small.tile([AC, 1], F32, tag="nmx")
        nc.vector.tensor_reduce(nmx[:], yg[:], axis=AX.X, op=OP.max, negate=True)
        S = p_small.tile([AC, 1], F32, tag="S")
        eu = p_small.tile([AC, V], F32, tag="euv")
        nc.scalar.activation(eu[:], yg[:], AF.Exp, bias=nmx[:], accum_out=S[:])
        q = p_small.tile([AC, 1], F32, tag="q")
        scr2 = p_small.tile([AC, V], F32, tag="scr2")
        nc.scalar.activation(scr2[:], eu[:], AF.Square, accum_out=q[:])
        rs = p_small.tile([AC, 1], F32, tag="rs")
        nc.vector.reciprocal(rs[:], S[:])
        tq = p_small.tile([AC, 1], F32, tag="tq")
        nc.vector.tensor_mul(tq[:], q[:], rs[:])
        nc.vector.tensor_mul(cost_buf[:AC, 8 * t + 4 : 8 * t + 5], tq[:], rs[:])

    # ================= time loop =============================================
    for t in range(T):
        u_path(t)
        sums_p = [
            p_sums.tile([128, AC], F32, tag=f"sp{k}", name=f"sums_p{k}_{t}")
            for k in range(2)
        ]
        p_path(t, sums_p)
        ubar = u_fc(t)
        mg = action_pre(t)
        action_post(t, sums_p, mg, ubar)

    # ---------------- final cost reduction ----------------
    csum = persist("csum", (128, 1))
    nc.vector.reduce_sum(csum[:], cost_buf[:], axis=AX.X)
    psc = psS.tile([1, 1], F32, tag="sm")
    mm(psc[:], bia[:, BONEC : BONEC + 1], csum[:], True, True)
    cost_sb = persist("cost_sb", (1, 1))
    nc.scalar.copy(cost_sb[:], psc[:])
    nc.sync.dma_start(cost_out[:], cost_sb[:])

    stack.close()


_CACHED = None


def _build():
    global _CACHED
    if _CACHED is not None:
        return _CACHED
    nc = bacc.Bacc("TRN2", target_bir_lowering=False, debug=False)
    D = {}
    for name, shape, dt_ in _DRAM_SPECS:
        D[name] = nc.dram_tensor(name, list(shape), dt_, kind="ExternalInput")
    cost_out = nc.dram_tensor("cost", [1, 1], F32, kind="ExternalOutput")
    with tile.TileContext(nc) as tc:
        _emit(tc, D, cost_out)
    nc.compile()
    _CACHED = nc
    return nc


def _host_inputs(core, inputs):
    f32 = np.float32
    bf16 = ml_dtypes.bfloat16
    i0, i1 = core * AC, (core + 1) * AC
    utter = np.asarray(inputs["utterances"], f32)
    obs = np.asarray(inputs["observations"], f32)[i0:i1]
    phys = np.asarray(inputs["physical"], f32)
    goals = np.asarray(inputs["observed_goals"], f32)[i0:i1]
    memu0 = np.asarray(inputs["mem_utterance"], f32)[0]     # [A, H], i-uniform
    memp = np.asarray(inputs["mem_physical"], f32)[i0:i1]
    mema = np.asarray(inputs["mem_action"], f32)[i0:i1]
    gumb = np.asarray(inputs["gumbel_u"], f32)[:, i0:i1]

    g = {k: np.asarray(inputs[k], f32) for k in (
        "u_Wih", "u_Whh", "u_bih", "u_bhh", "u_Wfc", "u_bfc",
        "gp_W1", "gp_b1", "gp_W2", "gp_b2",
        "p_Wih", "p_Whh", "p_bih", "p_bhh", "p_Wfc", "p_bfc",
        "g_Wih", "g_Whh", "g_bih", "g_bhh", "g_Wfc", "g_bfc",
        "a_Wih", "a_Whh", "a_bih", "a_bhh", "a_Wfc", "a_bfc",
        "m_W1", "m_b1", "m_W2", "m_b2", "uc_W1", "uc_b1", "uc_W2", "uc_b2")}

    m = {}
    # ---- P path ----
    xp = np.concatenate(
        [obs, np.broadcast_to(phys[None], (AC, E, PD))], axis=-1
    ).reshape(NP, OD + PD)
    m["xpT"] = np.ascontiguousarray(xp.T).astype(bf16)
    gi_n = xp @ g["p_Wih"][2 * H :].T + g["p_bih"][2 * H :]
    m["giPn"] = np.ascontiguousarray(gi_n.T).astype(bf16)
    m["hP0"] = np.ascontiguousarray(memp.reshape(NP, H).T).astype(bf16)
    m["whhP"] = np.ascontiguousarray(g["p_Whh"].T).astype(bf16)
    m["wihPrz"] = np.ascontiguousarray(g["p_Wih"][: 2 * H].T).astype(bf16)
    m["wfcP"] = np.ascontiguousarray(g["p_Wfc"].T).astype(bf16)
    m["bhnP"] = np.ascontiguousarray(g["p_bhh"][2 * H :][None, :]).astype(bf16)
    m["onesr"] = np.ones((1, 1024), bf16)

    # ---- U path (packed [128, 2*A]: chunk c at cols c*A) ----
    def packT(x):  # x [A(j), H] -> [128, 2*A]
        xT = np.ascontiguousarray(x.T)  # [H, A]
        return np.concatenate([xT[0:128], xT[128:256]], axis=1)

    m["hU0"] = packT(memu0).astype(bf16)
    gi_un = utter @ g["u_Wih"][2 * H :].T + g["u_bih"][2 * H :]   # [A, H]
    m["giUn"] = packT(gi_un).astype(bf16)
    m["xuT"] = np.ascontiguousarray(utter.T).astype(bf16)
    m["whhU"] = np.ascontiguousarray(g["u_Whh"].T).astype(bf16)
    m["wihUrz"] = np.ascontiguousarray(g["u_Wih"][: 2 * H].T).astype(bf16)
    m["wfcU"] = np.ascontiguousarray(g["u_Wfc"].T).astype(bf16)
    m["wgp1"] = np.ascontiguousarray(g["gp_W1"].T).astype(bf16)
    m["wgp2"] = np.ascontiguousarray(g["gp_W2"].T).astype(bf16)
    ur = np.zeros((1, URN), f32)
    ur[0, UR_RZ : UR_RZ + 512] = (g["u_bih"] + g["u_bhh"])[: 2 * H]
    ur[0, UR_BHN : UR_BHN + 256] = g["u_bhh"][2 * H :]
    ur[0, UR_FC : UR_FC + 256] = g["u_bfc"]
    ur[0, UR_GP1 : UR_GP1 + 256] = g["gp_b1"] - g["gp_W1"].sum(1)
    ur[0, UR_ONE : UR_ONE + 128] = 1.0
    m["urows"] = ur.astype(bf16)

    # ---- action module ----
    m["memaT"] = np.ascontiguousarray(mema.T).astype(bf16)
    ggi = goals @ g["g_Wih"].T + g["g_bih"]
    m["ggiT"] = np.ascontiguousarray(ggi.T)
    G = -np.log(-np.log(gumb + f32(EPS)) + f32(EPS)).astype(f32)
    m["gumb"] = np.ascontiguousarray(G.transpose(1, 0, 2).reshape(AC, T * V))
    m["whhG"] = np.ascontiguousarray(g["g_Whh"].T).astype(bf16)
    m["wfcG"] = np.ascontiguousarray(g["g_Wfc"].T).astype(bf16)
    wihA = g["a_Wih"].copy()
    wihA[:, :H] /= E          # phys channel carries sums (phys_feat+1)*E
    wihA[:, H : 2 * H] /= A   # utt channel carries sums (utt_feat+1)*A
    m["wihA"] = np.ascontiguousarray(wihA.T).astype(bf16)
    m["whhA"] = np.ascontiguousarray(g["a_Whh"].T).astype(bf16)
    m["wfcA"] = np.ascontiguousarray(g["a_Wfc"].T).astype(bf16)
    m["wm1"] = np.ascontiguousarray(g["m_W1"].T).astype(bf16)
    m["wm2"] = np.ascontiguousarray(g["m_W2"].T).astype(bf16)
    m["wuc1"] = np.ascontiguousarray(g["uc_W1"].T).astype(bf16)
    m["wuc2"] = np.ascontiguousarray(g["uc_W2"].T).astype(bf16)

    bias = np.zeros((128, NBIAS), f32)

    def putb(col, vec):
        vec = np.asarray(vec, f32).ravel()
        nt = (len(vec) + 127) // 128
        for k in range(nt):
            sl = vec[k * 128 : (k + 1) * 128]
            bias[: len(sl), col + k] = sl

    putb(BRZ_P, (g["p_bih"][: 2 * H] + g["p_bhh"][: 2 * H]) / 2)
    putb(BHN_P, g["p_bhh"][2 * H :])
    putb(BFC_P, g["p_bfc"])
    putb(BGP2, 4.0 * (g["gp_b2"] - g["gp_W2"].sum(1)))
    bias[:, BONEC] = 1.0
    putb(BNRZ_G, g["g_bhh"][: 2 * H] / 2)
    putb(BHN_G, g["g_bhh"][2 * H :])
    putb(BFC_G, g["g_bfc"])
    a_bih_adj = g["a_bih"] - g["a_Wih"].sum(1)
    putb(BNRZ_A, (a_bih_adj[: 2 * H] + g["a_bhh"][: 2 * H]) / 2)
    putb(BIN_A, a_bih_adj[2 * H :])
    putb(BHN_A, g["a_bhh"][2 * H :])
    putb(BFC_A, g["a_bfc"])
    putb(BM1, g["m_b1"] - g["m_W1"].sum(1))
    putb(BM2, g["m_b2"] - g["m_W2"].sum(1))
    putb(BUC1, g["uc_b1"] - g["uc_W1"].sum(1))
    bias[0, BUC2R : BUC2R + V] = (g["uc_b2"] - g["uc_W2"].sum(1)).astype(f32)
    bias[0, BONER : BONER + AC] = 1.0
    m["biases"] = bias
    return m


def kernel(**inputs) -> np.ndarray:
    nc = _build()
    in_maps = [_host_inputs(c, inputs) for c in range(NCORES)]
    res = run_bass_kernel_spmd(nc, in_maps, core_ids=list(range(NCORES)))
    total = np.float32(0.0)
    for r in res.results:
        total += np.float32(r["cost"].reshape(-1)[0])
    return np.array([total], np.float32)


if __name__ == "__main__":
    _build()
    print("build ok")


# revision 26
# speedup vs baseline: 1.2255x; 1.2255x over previous
"""Trainium2 Bass kernel for nn_AgentModule (multi-agent GRU game module).

Strategy v2 (collapsed utterance pathway):
 - mem_utterance is zero-filled and utt_x[i,j]=utterances[j] is independent of
   the receiving agent i, so the [A,A,H] utterance GRU state stays i-uniform
   for all T steps: the whole utterance pathway collapses exactly to [A,H].
   Each core computes the collapsed path (replicated, tiny) and scales the
   goal-prediction cost by AC agents via the Square activation scale.
 - Physical pathway shards agents across 8 cores (AC=16, NP=AC*E=4096 rows),
   feature-major layout h^T [H, NP] in SBUF bf16.
 - Gate evacuation touches each PSUM word once: rz gates leave PSUM through
   ACT tanh(0.5x+b/2) (sigmoid kept in tanh form); the static rank-5 input
   term is accumulated on the PE as an extra K-chunk; the n-gate static term
   giPn stays SBUF-resident and is folded in with one gpsimd add.
 - ELU decomposition elu(x)+1 = min(exp(x),1) + relu(x): per-agent pooled
   sums come from fused STT accum_out, no full reduce instructions.
 - One ACT table set (exp_and_others: tanh/exp/relu/square) for the whole
   kernel; engine balance PE/ACT/DVE/GPSIMD tuned per cost model.
"""

import sys

for _p in ("/opt/trn_rl_repo", "/opt/pypackages"):
    if _p not in sys.path:
        sys.path.append(_p)

import numpy as np
import ml_dtypes

import concourse.bass as bass
import concourse.bacc as bacc
import concourse.mybir as mybir
import concourse.tile as tile
from concourse.bass_utils import run_bass_kernel_spmd

F32 = mybir.dt.float32
BF16 = mybir.dt.bfloat16
AF = mybir.ActivationFunctionType
OP = mybir.AluOpType
AX = mybir.AxisListType

NCORES = 8
A = 128
L = 128
E = A + L          # 256
T = 8
H = 256
V = 32
GS = 5
GI = 5
OD = 2
PD = 3
MD = 2
STEP = 0.1
EPS = 1e-20
AC = A // NCORES   # 16 agents per core
NP = AC * E        # 4096 physical rows per core
CB = 1024          # column-block width for the P path
NCB = NP // CB     # 4

# bias matrix columns
BRZ_P = 0     # 4 cols: (p_bih+p_bhh)[:512]/2
BHN_P = 4     # 2 cols: p_bhh[512:768]
BFC_P = 6     # 2 cols: p_bfc
BGP2 = 8      # 1 col (5 rows): 4*(gp_b2 - gp_W2.sum(1))
BONEC = 9     # ones column
BNRZ_G = 10   # 4 cols: g_bhh[:512]/2
BHN_G = 14    # 2 cols: g_bhh[512:768]
BFC_G = 16    # 2 cols: g_bfc
BNRZ_A = 18   # 4 cols: (a_bih_adj[:512]+a_bhh[:512])/2
BIN_A = 22    # 2 cols: a_bih_adj[512:768]
BHN_A = 24    # 2 cols: a_bhh[512:768]
BFC_A = 26    # 2 cols: a_bfc
BM1 = 28      # 2 cols: m_b1 - m_W1.sum(1)
BM2 = 30     # 1 col (2 rows): m_b2 - m_W2.sum(1)
BUC1 = 31     # 2 cols: uc_b1 - uc_W1.sum(1)
BUC2R = 33    # row0 cols 33:65: uc_b2 - uc_W2.sum(1)
BONER = 65    # row0 cols 65:81: ones(16)
NBIAS = 82

# urows segments (bf16 bias rows for the U path PE bias-passes)
UR_RZ = 0       # 512: (u_bih+u_bhh)[:512]
UR_BHN = 512    # 256: u_bhh[512:768]
UR_FC = 768     # 256: u_bfc
UR_GP1 = 1024   # 256: gp_b1 - gp_W1.sum(1)
UR_ONE = 1280   # 128: ones
URN = 1408

_DRAM_SPECS = [
    # P path
    ("hP0", (H, NP), BF16),
    ("giPn", (H, NP), BF16),
    ("xpT", (OD + PD, NP), BF16),
    ("whhP", (H, 3 * H), BF16),
    ("wihPrz", (OD + PD, 2 * H), BF16),
    ("wfcP", (H, H), BF16),
    ("bhnP", (1, H), BF16),
    ("onesr", (1, 1024), BF16),
    # U path (collapsed, replicated; packed [128, 2*128] layout)
    ("hU0", (128, 2 * A), BF16),
    ("giUn", (128, 2 * A), BF16),
    ("xuT", (V, A), BF16),
    ("whhU", (H, 3 * H), BF16),
    ("wihUrz", (V, 2 * H), BF16),
    ("wfcU", (H, H), BF16),
    ("wgp1", (H, H), BF16),
    ("wgp2", (H, GS), BF16),
    ("urows", (1, URN), BF16),
    # action module
    ("memaT", (H, AC), BF16),
    ("ggiT", (3 * H, AC), F32),
    ("gumb", (AC, T * V), F32),
    ("whhG", (H, 3 * H), BF16),
    ("wfcG", (H, H), BF16),
    ("wihA", (3 * H, 3 * H), BF16),
    ("whhA", (H, 3 * H), BF16),
    ("wfcA", (H, H), BF16),
    ("wm1", (H, H), BF16),
    ("wm2", (H, MD), BF16),
    ("wuc1", (H, H), BF16),
    ("wuc2", (H, V), BF16),
    ("biases", (128, NBIAS), F32),
]


def _emit(tc, D, cost_out):
    nc = tc.nc
    import contextlib

    stack = contextlib.ExitStack()
    pers = stack.enter_context(tc.tile_pool(name="pers", bufs=1))

    def persist(name, shape, dtype=F32):
        return pers.tile(list(shape), dtype, tag=name, name=name)

    def load2(name, rows, cols, dtype=F32, ptile=128):
        nt = (rows + ptile - 1) // ptile
        out = []
        for k in range(nt):
            p = min(ptile, rows - k * ptile)
            tl = persist(f"{name}_{k}", (p, cols), dtype)
            nc.sync.dma_start(tl[:], D[name][k * ptile : k * ptile + p, :])
            out.append(tl)
        return out

    # ---------------- persistent state + weights ----------------
    hP = load2("hP0", H, NP, dtype=BF16)          # 2 x [128, 4096]
    giPn = load2("giPn", H, NP, dtype=BF16)       # 2 x [128, 4096]
    xpT = load2("xpT", OD + PD, NP, dtype=BF16)[0]
    whhP = load2("whhP", H, 3 * H, dtype=BF16)
    wihPrz = load2("wihPrz", OD + PD, 2 * H, dtype=BF16)[0]
    wfcP = load2("wfcP", H, H, dtype=BF16)
    bhnP = load2("bhnP", 1, H, dtype=BF16)[0]
    onesr = load2("onesr", 1, 1024, dtype=BF16)[0]

    hU = load2("hU0", 128, 2 * A, dtype=BF16)[0]  # packed [128, 256]
    giUn = load2("giUn", 128, 2 * A, dtype=BF16)[0]
    xuT = load2("xuT", V, A, dtype=BF16)[0]
    whhU = load2("whhU", H, 3 * H, dtype=BF16)
    wihUrz = load2("wihUrz", V, 2 * H, dtype=BF16)[0]
    wfcU = load2("wfcU", H, H, dtype=BF16)
    wgp1 = load2("wgp1", H, H, dtype=BF16)
    wgp2 = load2("wgp2", H, GS, dtype=BF16)
    urows = load2("urows", 1, URN, dtype=BF16)[0]

    hA = load2("memaT", H, AC, dtype=BF16)
    ggiT = load2("ggiT", 3 * H, AC)
    gumb = load2("gumb", AC, T * V)[0]
    whhG = load2("whhG", H, 3 * H, dtype=BF16)
    wfcG = load2("wfcG", H, H, dtype=BF16)
    wihA = load2("wihA", 3 * H, 3 * H, dtype=BF16)
    whhA = load2("whhA", H, 3 * H, dtype=BF16)
    wfcA = load2("wfcA", H, H, dtype=BF16)
    wm1 = load2("wm1", H, H, dtype=BF16)
    wm2 = load2("wm2", H, MD, dtype=BF16)
    wuc1 = load2("wuc1", H, H, dtype=BF16)
    wuc2 = load2("wuc2", H, V, dtype=BF16)
    bia = load2("biases", 128, NBIAS)[0]

    def bvec(idx, p=128):
        return bia[:p, idx : idx + 1]

    cost_buf = persist("cost_buf", (128, 8 * T))
    nc.vector.memset(cost_buf[:], 0.0)
    ones16 = persist("ones16", (128, AC), BF16)
    nc.vector.memset(ones16[:], 1.0)

    # ---------------- pools ----------------
    psG = stack.enter_context(tc.tile_pool(name="psG", bufs=3, space="PSUM"))
    psS = stack.enter_context(tc.tile_pool(name="psS", bufs=2, space="PSUM"))
    psF = psG
    p_tr = stack.enter_context(tc.tile_pool(name="p_tr", bufs=4))
    p_tz = stack.enter_context(tc.tile_pool(name="p_tz", bufs=8))
    p_rf = stack.enter_context(tc.tile_pool(name="p_rf", bufs=8))
    p_t1 = stack.enter_context(tc.tile_pool(name="p_t1", bufs=3))
    p_t2 = stack.enter_context(tc.tile_pool(name="p_t2", bufs=3))
    p_n = stack.enter_context(tc.tile_pool(name="p_n", bufs=8))
    p_d = stack.enter_context(tc.tile_pool(name="p_d", bufs=3))
    p_e = stack.enter_context(tc.tile_pool(name="p_e", bufs=3))
    p_e1 = stack.enter_context(tc.tile_pool(name="p_e1", bufs=3))
    p_r1 = stack.enter_context(tc.tile_pool(name="p_r1", bufs=3))
    p_scr = stack.enter_context(tc.tile_pool(name="p_scr", bufs=4))
    p_u = stack.enter_context(tc.tile_pool(name="p_u", bufs=2))
    p_ubar = stack.enter_context(tc.tile_pool(name="p_ubar", bufs=2))
    p_sums = stack.enter_context(tc.tile_pool(name="p_sums", bufs=2))
    p_small = stack.enter_context(tc.tile_pool(name="p_small", bufs=2))

    def mm(out, lhsT, rhs, start, stop):
        nc.tensor.matmul(out, lhsT, rhs, start=start, stop=stop)

    # ================= U path (collapsed utterance + goal pred) =============
    def u_path(t):
        # gates: rz packed [128, 512] (4 Mtiles of 128 agents)
        psu = psS.tile([128, 4 * A], F32, tag="sm", name=f"psu_{t}")
        for m in range(4):
            sl = psu[:, m * A : (m + 1) * A]
            mm(sl, whhU[0][:, m * 128 : m * 128 + 128], hU[:, 0:A], True, False)
            mm(sl, whhU[1][:, m * 128 : m * 128 + 128], hU[:, A : 2 * A], False, False)
            mm(sl, wihUrz[:, m * 128 : m * 128 + 128], xuT[:], False, False)
            mm(sl, urows[:, UR_RZ + m * 128 : UR_RZ + m * 128 + 128],
               urows[:, UR_ONE : UR_ONE + A], False, True)
        trzu = p_u.tile([128, 4 * A], BF16, tag="trzu", name=f"trzu_{t}")
        nc.scalar.activation(trzu[:], psu[:], AF.Tanh, scale=0.5)
        rfu = p_u.tile([128, 2 * A], BF16, tag="rfu", name=f"rfu_{t}")
        nc.vector.tensor_scalar(rfu[:], trzu[:, 0 : 2 * A], 0.5, 0.5, OP.mult, OP.add)
        # n gates packed [128, 256] (+ bhn bias-pass, gi stays outside)
        psn = psS.tile([128, 2 * A], F32, tag="sm", name=f"psn_{t}")
        for k in range(2):
            m = 4 + k
            sl = psn[:, k * A : (k + 1) * A]
            mm(sl, whhU[0][:, m * 128 : m * 128 + 128], hU[:, 0:A], True, False)
            mm(sl, whhU[1][:, m * 128 : m * 128 + 128], hU[:, A : 2 * A], False, False)
            mm(sl, urows[:, UR_BHN + k * 128 : UR_BHN + k * 128 + 128],
               urows[:, UR_ONE : UR_ONE + A], False, True)
        t1u = p_u.tile([128, 2 * A], BF16, tag="t1u", name=f"t1u_{t}")
        nc.vector.tensor_mul(t1u[:], psn[:], rfu[:])
        t2u = p_u.tile([128, 2 * A], BF16, tag="t2u", name=f"t2u_{t}")
        nc.vector.tensor_add(t2u[:], t1u[:], giUn[:])
        nu = p_u.tile([128, 2 * A], BF16, tag="nu", name=f"nu_{t}")
        nc.scalar.activation(nu[:], t2u[:], AF.Tanh)
        du = p_u.tile([128, 2 * A], BF16, tag="du", name=f"du_{t}")
        nc.vector.tensor_sub(du[:], hU[:], nu[:])
        eu = p_u.tile([128, 2 * A], BF16, tag="eu", name=f"eu_{t}")
        nc.vector.scalar_tensor_tensor(eu[:], trzu[:, 2 * A : 4 * A], 1.0, du[:],
                                       OP.add, OP.mult)
        nc.vector.scalar_tensor_tensor(hU[:], eu[:], 0.5, nu[:], OP.mult, OP.add)

    def u_fc(t):
        # fc -> m_u (= utt_proc+1) + ubar accum
        psf = psS.tile([128, 2 * A], F32, tag="sm", name=f"psf_{t}")
        for mf in range(2):
            sl = psf[:, mf * A : (mf + 1) * A]
            mm(sl, wfcU[0][:, mf * 128 : mf * 128 + 128], hU[:, 0:A], True, False)
            mm(sl, wfcU[1][:, mf * 128 : mf * 128 + 128], hU[:, A : 2 * A], False, False)
            mm(sl, urows[:, UR_FC + mf * 128 : UR_FC + mf * 128 + 128],
               urows[:, UR_ONE : UR_ONE + A], False, True)
        e1u = p_u.tile([128, 2 * A], BF16, tag="e1u", name=f"e1u_{t}")
        nc.scalar.activation(e1u[:], psf[:], AF.Exp)
        r1u = p_u.tile([128, 2 * A], BF16, tag="r1u", name=f"r1u_{t}")
        nc.vector.tensor_scalar(r1u[:], psf[:], 0.0, 1.0, OP.max, OP.add)
        mu = p_u.tile([128, 2 * A], BF16, tag="mu", name=f"mu_{t}")
        ubar = []
        for c in range(2):
            ub_c = p_ubar.tile([128, 1], F32, tag=f"ubar{c}", name=f"ubar{c}_{t}")
            sl = slice(c * A, (c + 1) * A)
            nc.vector.scalar_tensor_tensor(
                mu[:, sl], e1u[:, sl], 1.0, r1u[:, sl], OP.mult, OP.min,
                accum_out=ub_c[:],
            )
            ubar.append(ub_c)
        # goal prediction: gp1 (elu, shifted) -> gp2 -> squared cost
        psg = psS.tile([128, 2 * A], F32, tag="sm", name=f"psg_{t}")
        for mf in range(2):
            sl = psg[:, mf * A : (mf + 1) * A]
            mm(sl, wgp1[0][:, mf * 128 : mf * 128 + 128], mu[:, 0:A], True, False)
            mm(sl, wgp1[1][:, mf * 128 : mf * 128 + 128], mu[:, A : 2 * A], False, False)
            mm(sl, urows[:, UR_GP1 + mf * 128 : UR_GP1 + mf * 128 + 128],
               urows[:, UR_ONE : UR_ONE + A], False, True)
        e1g = p_u.tile([128, 2 * A], BF16, tag="e1g", name=f"e1g_{t}")
        nc.scalar.activation(e1g[:], psg[:], AF.Exp)
        r1g = p_u.tile([128, 2 * A], BF16, tag="r1g", name=f"r1g_{t}")
        nc.vector.tensor_scalar(r1g[:], psg[:], 0.0, 1.0, OP.max, OP.add)
        y1 = p_u.tile([128, 2 * A], BF16, tag="y1", name=f"y1_{t}")
        nc.vector.scalar_tensor_tensor(y1[:], e1g[:], 1.0, r1g[:], OP.mult, OP.min)
        ps5 = psS.tile([GS, A], F32, tag="sm", name=f"ps5_{t}")
        mm(ps5[:], wgp2[0][:, :], y1[:, 0:A], True, False)
        mm(ps5[:], wgp2[1][:, :], y1[:, A : 2 * A], False, True)
        sq = p_u.tile([GS, A], BF16, tag="sq", name=f"sq_{t}")
        # cost += AC * sum(goal_pred^2): (4x+4b')^2 = 16 (x+b')^2
        nc.scalar.activation(
            sq[:], ps5[:], AF.Square, bias=bvec(BGP2, GS), scale=4.0,
            accum_out=cost_buf[:GS, 8 * t : 8 * t + 1],
        )
        return ubar

    # ================= P path (physical processor) ===========================
    def p_path(t, sums_p):
        trz = {}
        rfull = {}
        nts = {}
        for m in (2, 3, 0, 1, 4, 5):
            units = []
            for cb in range(NCB):
                u = psG.tile([128, CB], F32, tag="gps", name=f"gps_{t}_{m}_{cb}")
                units.append(u)
            is_n = m >= 4
            nk = 3 if not is_n else 2
            for k in range(2):
                lhsT = whhP[k][:, m * 128 : m * 128 + 128]
                for cb in range(NCB):
                    for s in range(2):
                        mm(units[cb][:, s * 512 : (s + 1) * 512], lhsT,
                           hP[k][:, cb * CB + s * 512 : cb * CB + s * 512 + 512],
                           k == 0, k == nk - 1)
            if not is_n:
                lhsT = wihPrz[:, m * 128 : m * 128 + 128]
                for cb in range(NCB):
                    for s in range(2):
                        mm(units[cb][:, s * 512 : (s + 1) * 512], lhsT,
                           xpT[:, cb * CB + s * 512 : cb * CB + s * 512 + 512],
                           False, True)
            if m < 2:           # r gates: tanh form then sigmoid affine
                rfull[m] = []
                for cb in range(NCB):
                    tr = p_tr.tile([128, CB], BF16, tag="tr")
                    nc.scalar.activation(tr[:], units[cb][:], AF.Tanh,
                                         bias=bvec(BRZ_P + m), scale=0.5)
                    rf = p_rf.tile([128, CB], BF16, tag="rf",
                                   name=f"rf_{t}_{m}_{cb}")
                    nc.vector.tensor_scalar(rf[:], tr[:], 0.5, 0.5, OP.mult, OP.add)
                    rfull[m].append(rf)
            elif m >= 4:        # n gates
                k2 = m - 4
                nts[k2] = []
                for cb in range(NCB):
                    t1 = p_t1.tile([128, CB], BF16, tag="t1")
                    nc.vector.scalar_tensor_tensor(
                        t1[:], units[cb][:], bvec(BHN_P + k2), rfull[k2][cb][:],
                        OP.add, OP.mult)
                    t2 = p_t2.tile([128, CB], BF16, tag="t2")
                    nc.gpsimd.tensor_add(
                        t2[:], t1[:], giPn[k2][:, cb * CB : (cb + 1) * CB])
                    n_ = p_n.tile([128, CB], BF16, tag="n",
                                  name=f"n_{t}_{k2}_{cb}")
                    nc.scalar.activation(n_[:], t2[:], AF.Tanh)
                    nts[k2].append(n_)
            else:               # z gates -> sigmoid form
                trz[m - 2] = []
                for cb in range(NCB):
                    tzt = p_tr.tile([128, CB], BF16, tag="tr")
                    nc.scalar.activation(tzt[:], units[cb][:], AF.Tanh,
                                         bias=bvec(BRZ_P + m), scale=0.5)
                    tz = p_tz.tile([128, CB], BF16, tag="tz",
                                   name=f"tz_{t}_{m}_{cb}")
                    nc.vector.tensor_scalar(tz[:], tzt[:], 0.5, 0.5,
                                            OP.mult, OP.add)
                    trz[m - 2].append(tz)
        # blend: h' = n + 0.5(1+tz)(h-n)
        for k in range(2):
            for cb in range(NCB):
                hk = hP[k][:, cb * CB : (cb + 1) * CB]
                d = p_d.tile([128, CB], BF16, tag="d")
                nc.gpsimd.tensor_sub(d[:], hk, nts[k][cb][:])
                e_ = p_e.tile([128, CB], BF16, tag="e")
                nc.vector.tensor_mul(e_[:], trz[k][cb][:], d[:])
                nc.vector.tensor_add(hk, e_[:], nts[k][cb][:])
        # fc -> per-agent pooled sums (elu+1 = min(exp,1) + relu)
        for mf in range(2):
            funits = []
            for cb in range(NCB):
                u = psG.tile([128, CB], F32, tag="gps",
                             name=f"fps_{t}_{mf}_{cb}")
                funits.append(u)
            for k in range(2):
                lhsT = wfcP[k][:, mf * 128 : mf * 128 + 128]
                for cb in range(NCB):
                    for s in range(2):
                        mm(funits[cb][:, s * 512 : (s + 1) * 512], lhsT,
                           hP[k][:, cb * CB + s * 512 : cb * CB + s * 512 + 512],
                           k == 0, k == 1)
            for cb in range(NCB):
                e1 = p_e1.tile([128, CB], BF16, tag="e1")
                nc.scalar.activation(e1[:], funits[cb][:], AF.Exp,
                                     bias=bvec(BFC_P + mf))
                r1 = p_r1.tile([128, CB], BF16, tag="r1")
                nc.scalar.activation(r1[:], funits[cb][:], AF.Relu,
                                     bias=bvec(BFC_P + mf))
                for a in range(CB // E):
                    sl = slice(a * E, (a + 1) * E)
                    scr = p_scr.tile([128, E], BF16, tag="scr")
                    ai = cb * (CB // E) + a
                    nc.vector.scalar_tensor_tensor(
                        scr[:], e1[:, sl], 1.0, r1[:, sl], OP.min, OP.add,
                        accum_out=sums_p[mf][:, ai : ai + 1])

    # ================= action module =========================================
    def small_gru_gates(whh, hT, m, extra_k=None):
        ps = psS.tile([128, AC], F32, tag="sm")
        first = True
        if extra_k is not None:
            for ki, rhs in enumerate(extra_k):
                mm(ps[:], wihA[ki][:, m * 128 : m * 128 + 128], rhs[:], first, False)
                first = False
        mm(ps[:], whh[0][:, m * 128 : m * 128 + 128], hT[0][:], first, False)
        mm(ps[:], whh[1][:, m * 128 : m * 128 + 128], hT[1][:], False, True)
        return ps

    def small_fc(wfc, rhs, tag="sm"):
        out = []
        for mf in range(2):
            ps = psS.tile([128, AC], F32, tag=tag)
            mm(ps[:], wfc[0][:, mf * 128 : mf * 128 + 128], rhs[0][:], True, False)
            mm(ps[:], wfc[1][:, mf * 128 : mf * 128 + 128], rhs[1][:], False, True)
            out.append(ps)
        return out

    def exp_sigmoid(in_ap, hbidx, name=None):
        th = p_small.tile([128, AC], F32, tag="es", name=name)
        nc.scalar.activation(th[:], in_ap, AF.Tanh, bias=bvec(hbidx), scale=0.5)
        s_ = p_small.tile([128, AC], BF16, tag="es3", name=(name or "") + "s")
        nc.vector.tensor_scalar(s_[:], th[:], 0.5, 0.5, OP.mult, OP.add)
        return s_

    def elu_shift_small(ps, bidx, tag, p=128):
        e1 = p_small.tile([p, AC], BF16, tag=tag + "e")
        nc.scalar.activation(e1[:], ps[:], AF.Exp, bias=bvec(bidx, p))
        r1 = p_small.tile([p, AC], BF16, tag=tag + "r")
        nc.scalar.activation(r1[:], ps[:], AF.Relu, bias=bvec(bidx, p))
        m_ = p_small.tile([p, AC], BF16, tag=tag + "m")
        nc.vector.scalar_tensor_tensor(m_[:], r1[:], 1.0, e1[:], OP.add, OP.min)
        return m_

    def small_elu(ps_pair, bidx, tag):
        return [elu_shift_small(ps_pair[mf], bidx + mf, f"{tag}{mf}")
                for mf in range(2)]

    def action_pre(t):
        # goal processor GRU (state not persisted)
        grz = []
        for m in range(4):
            ps = small_gru_gates(whhG, hA, m)
            tt = p_small.tile([128, AC], F32, tag="gt")
            nc.vector.tensor_add(tt[:], ps[:], ggiT[m][:])
            grz.append(exp_sigmoid(tt[:], BNRZ_G + m, name=f"grz{m}_{t}"))
        gn = []
        for k in range(2):
            m = 4 + k
            ps = small_gru_gates(whhG, hA, m)
            t1 = p_small.tile([128, AC], BF16, tag="gt")
            nc.vector.scalar_tensor_tensor(
                t1[:], ps[:], bvec(BHN_G + k), grz[k][:], OP.add, OP.mult)
            t2 = p_small.tile([128, AC], BF16, tag="gt2")
            nc.vector.tensor_add(t2[:], t1[:], ggiT[m][:])
            n_ = p_small.tile([128, AC], BF16, tag="gn")
            nc.scalar.activation(n_[:], t2[:], AF.Tanh)
            gn.append(n_)
        g2 = []
        for k in range(2):
            d = p_small.tile([128, AC], BF16, tag="gd")
            nc.vector.tensor_sub(d[:], hA[k][:], gn[k][:])
            e_ = p_small.tile([128, AC], BF16, tag="ge")
            nc.vector.tensor_mul(e_[:], grz[2 + k][:], d[:])
            g2k = p_small.tile([128, AC], BF16, tag="g2")
            nc.vector.tensor_add(g2k[:], gn[k][:], e_[:])
            g2.append(g2k)
        mg = small_elu(small_fc(wfcG, g2), BFC_G, "mg")
        return mg

    def action_post(t, sums_p, mg, ubar):
        # broadcast utterance feature sums to AC columns
        ub = []
        for c in range(2):
            u = p_small.tile([128, AC], BF16, tag=f"ub{c}", name=f"ub{c}_{t}")
            nc.vector.tensor_scalar_mul(u[:], ones16[:], ubar[c][:, 0:1])
            ub.append(u)
        xch = []
        for xi, src_t in enumerate(
            [sums_p[0], sums_p[1], ub[0], ub[1], mg[0], mg[1]]
        ):
            xb = p_small.tile([128, AC], BF16, tag=f"xb{xi}", name=f"xb{xi}_{t}")
            nc.vector.tensor_copy(xb[:], src_t[:])
            xch.append(xb)
        arz = []
        for m in range(4):
            ps = small_gru_gates(whhA, hA, m, extra_k=xch)
            arz.append(exp_sigmoid(ps[:], BNRZ_A + m, name=f"arz{m}_{t}"))
        an = []
        for k in range(2):
            m = 4 + k
            psg = psS.tile([128, AC], F32, tag="sm")
            for ki, rhs in enumerate(xch):
                mm(psg[:], wihA[ki][:, m * 128 : m * 128 + 128], rhs[:],
                   ki == 0, ki == 5)
            psh = psS.tile([128, AC], F32, tag="sm")
            mm(psh[:], whhA[0][:, m * 128 : m * 128 + 128], hA[0][:], True, False)
            mm(psh[:], whhA[1][:, m * 128 : m * 128 + 128], hA[1][:], False, True)
            t1 = p_small.tile([128, AC], BF16, tag="at1")
            nc.vector.scalar_tensor_tensor(
                t1[:], psh[:], bvec(BHN_A + k), arz[k][:], OP.add, OP.mult)
            t2 = p_small.tile([128, AC], F32, tag="at2")
            nc.vector.scalar_tensor_tensor(
                t2[:], psg[:], bvec(BIN_A + k), t1[:], OP.add, OP.add)
            n_ = p_small.tile([128, AC], BF16, tag="an")
            nc.scalar.activation(n_[:], t2[:], AF.Tanh)
            an.append(n_)
        for k in range(2):
            d = p_small.tile([128, AC], BF16, tag="ad")
            nc.vector.tensor_sub(d[:], hA[k][:], an[k][:])
            e_ = p_small.tile([128, AC], BF16, tag="ae")
            nc.vector.tensor_mul(e_[:], arz[2 + k][:], d[:])
            nc.vector.tensor_add(hA[k][:], an[k][:], e_[:])
        mp = small_elu(small_fc(wfcA, hA), BFC_A, "mp")

        # movement chooser -> cost
        ym = small_elu(small_fc(wm1, mp), BM1, "ym")
        psmv = psS.tile([MD, AC], F32, tag="sm")
        mm(psmv[:], wm2[0][:, :], ym[0][:], True, False)
        mm(psmv[:], wm2[1][:, :], ym[1][:], False, True)
        mvt = p_small.tile([MD, AC], F32, tag="mvt")
        nc.scalar.activation(mvt[:], psmv[:], AF.Tanh, bias=bvec(BM2, MD))
        fmv = p_small.tile([MD, AC], F32, tag="fmv")
        nc.vector.tensor_scalar(fmv[:], mvt[:], 2.0 * STEP, -STEP, OP.mult, OP.add)
        scr = p_small.tile([MD, AC], F32, tag="mscr")
        nc.scalar.activation(
            scr[:], fmv[:], AF.Square,
            accum_out=cost_buf[:MD, 8 * t + 5 : 8 * t + 6])

        # utterance chooser -> gumbel softmax -> cost
        yu = small_elu(small_fc(wuc1, mp), BUC1, "yu")
        pslg = psS.tile([AC, V], F32, tag="sm")
        mm(pslg[:], yu[0][:], wuc2[0][:, :], True, False)
        mm(pslg[:], yu[1][:], wuc2[1][:, :], False, False)
        mm(pslg[:], bia[:1, BONER : BONER + AC], bia[:1, BUC2R : BUC2R + V],
           False, True)
        yg = p_small.tile([AC, V], F32, tag="yg")
        nc.vector.tensor_add(yg[:], pslg[:], gumb[:, t * V : (t + 1) * V])
        nmx = p_small.tile([AC, 1], F32, tag="nmx")
        nc.vector.tensor_reduce(nmx[:], yg[:], axis=AX.X, op=OP.max, negate=True)
        S = p_small.tile([AC, 1], F32, tag="S")
        eu = p_small.tile([AC, V], F32, tag="euv")
        nc.scalar.activation(eu[:], yg[:], AF.Exp, bias=nmx[:], accum_out=S[:])
        q = p_small.tile([AC, 1], F32, tag="q")
        scr2 = p_small.tile([AC, V], F32, tag="scr2")
        nc.scalar.activation(scr2[:], eu[:], AF.Square, accum_out=q[:])
        rs = p_small.tile([AC, 1], F32, tag="rs")
        nc.vector.reciprocal(rs[:], S[:])
        tq = p_small.tile([AC, 1], F32, tag="tq")
        nc.vector.tensor_mul(tq[:], q[:], rs[:])
        nc.vector.tensor_mul(cost_buf[:AC, 8 * t + 4 : 8 * t + 5], tq[:], rs[:])

    # ================= time loop =============================================
    for t in range(T):
        u_path(t)
        sums_p = [
            p_sums.tile([128, AC], F32, tag=f"sp{k}", name=f"sums_p{k}_{t}")
            for k in range(2)
        ]
        mg = action_pre(t)
        p_path(t, sums_p)
        ubar = u_fc(t)
        action_post(t, sums_p, mg, ubar)

    # ---------------- final cost reduction ----------------
    csum = persist("csum", (128, 1))
    nc.vector.reduce_sum(csum[:], cost_buf[:], axis=AX.X)
    psc = psS.tile([1, 1], F32, tag="sm")
    mm(psc[:], bia[:, BONEC : BONEC + 1], csum[:], True, True)
    cost_sb = persist("cost_sb", (1, 1))
    nc.scalar.copy(cost_sb[:], psc[:])
    nc.sync.dma_start(cost_out[:], cost_sb[:])

    stack.close()


_CACHED = None


def _build():
    global _CACHED
    if _CACHED is not None:
        return _CACHED
    nc = bacc.Bacc("TRN2", target_bir_lowering=False, debug=False)
    D = {}
    for name, shape, dt_ in _DRAM_SPECS:
        D[name] = nc.dram_tensor(name, list(shape), dt_, kind="ExternalInput")
    cost_out = nc.dram_tensor("cost", [1, 1], F32, kind="ExternalOutput")
    with tile.TileContext(nc) as tc:
        _emit(tc, D, cost_out)
    nc.compile()
    _CACHED = nc
    return nc


def _host_inputs(core, inputs):
    f32 = np.float32
    bf16 = ml_dtypes.bfloat16
    i0, i1 = core * AC, (core + 1) * AC
    utter = np.asarray(inputs["utterances"], f32)
    obs = np.asarray(inputs["observations"], f32)[i0:i1]
    phys = np.asarray(inputs["physical"], f32)
    goals = np.asarray(inputs["observed_goals"], f32)[i0:i1]
    memu0 = np.asarray(inputs["mem_utterance"], f32)[0]     # [A, H], i-uniform
    memp = np.asarray(inputs["mem_physical"], f32)[i0:i1]
    mema = np.asarray(inputs["mem_action"], f32)[i0:i1]
    gumb = np.asarray(inputs["gumbel_u"], f32)[:, i0:i1]

    g = {k: np.asarray(inputs[k], f32) for k in (
        "u_Wih", "u_Whh", "u_bih", "u_bhh", "u_Wfc", "u_bfc",
        "gp_W1", "gp_b1", "gp_W2", "gp_b2",
        "p_Wih", "p_Whh", "p_bih", "p_bhh", "p_Wfc", "p_bfc",
        "g_Wih", "g_Whh", "g_bih", "g_bhh", "g_Wfc", "g_bfc",
        "a_Wih", "a_Whh", "a_bih", "a_bhh", "a_Wfc", "a_bfc",
        "m_W1", "m_b1", "m_W2", "m_b2", "uc_W1", "uc_b1", "uc_W2", "uc_b2")}

    m = {}
    # ---- P path ----
    xp = np.concatenate(
        [obs, np.broadcast_to(phys[None], (AC, E, PD))], axis=-1
    ).reshape(NP, OD + PD)
    m["xpT"] = np.ascontiguousarray(xp.T).astype(bf16)
    gi_n = xp @ g["p_Wih"][2 * H :].T + g["p_bih"][2 * H :]
    m["giPn"] = np.ascontiguousarray(gi_n.T).astype(bf16)
    m["hP0"] = np.ascontiguousarray(memp.reshape(NP, H).T).astype(bf16)
    m["whhP"] = np.ascontiguousarray(g["p_Whh"].T).astype(bf16)
    m["wihPrz"] = np.ascontiguousarray(g["p_Wih"][: 2 * H].T).astype(bf16)
    m["wfcP"] = np.ascontiguousarray(g["p_Wfc"].T).astype(bf16)
    m["bhnP"] = np.ascontiguousarray(g["p_bhh"][2 * H :][None, :]).astype(bf16)
    m["onesr"] = np.ones((1, 1024), bf16)

    # ---- U path (packed [128, 2*A]: chunk c at cols c*A) ----
    def packT(x):  # x [A(j), H] -> [128, 2*A]
        xT = np.ascontiguousarray(x.T)  # [H, A]
        return np.concatenate([xT[0:128], xT[128:256]], axis=1)

    m["hU0"] = packT(memu0).astype(bf16)
    gi_un = utter @ g["u_Wih"][2 * H :].T + g["u_bih"][2 * H :]   # [A, H]
    m["giUn"] = packT(gi_un).astype(bf16)
    m["xuT"] = np.ascontiguousarray(utter.T).astype(bf16)
    m["whhU"] = np.ascontiguousarray(g["u_Whh"].T).astype(bf16)
    m["wihUrz"] = np.ascontiguousarray(g["u_Wih"][: 2 * H].T).astype(bf16)
    m["wfcU"] = np.ascontiguousarray(g["u_Wfc"].T).astype(bf16)
    m["wgp1"] = np.ascontiguousarray(g["gp_W1"].T).astype(bf16)
    m["wgp2"] = np.ascontiguousarray(g["gp_W2"].T).astype(bf16)
    ur = np.zeros((1, URN), f32)
    ur[0, UR_RZ : UR_RZ + 512] = (g["u_bih"] + g["u_bhh"])[: 2 * H]
    ur[0, UR_BHN : UR_BHN + 256] = g["u_bhh"][2 * H :]
    ur[0, UR_FC : UR_FC + 256] = g["u_bfc"]
    ur[0, UR_GP1 : UR_GP1 + 256] = g["gp_b1"] - g["gp_W1"].sum(1)
    ur[0, UR_ONE : UR_ONE + 128] = 1.0
    m["urows"] = ur.astype(bf16)

    # ---- action module ----
    m["memaT"] = np.ascontiguousarray(mema.T).astype(bf16)
    ggi = goals @ g["g_Wih"].T + g["g_bih"]
    m["ggiT"] = np.ascontiguousarray(ggi.T)
    G = -np.log(-np.log(gumb + f32(EPS)) + f32(EPS)).astype(f32)
    m["gumb"] = np.ascontiguousarray(G.transpose(1, 0, 2).reshape(AC, T * V))
    m["whhG"] = np.ascontiguousarray(g["g_Whh"].T).astype(bf16)
    m["wfcG"] = np.ascontiguousarray(g["g_Wfc"].T).astype(bf16)
    wihA = g["a_Wih"].copy()
    wihA[:, :H] /= E          # phys channel carries sums (phys_feat+1)*E
    wihA[:, H : 2 * H] /= A   # utt channel carries sums (utt_feat+1)*A
    m["wihA"] = np.ascontiguousarray(wihA.T).astype(bf16)
    m["whhA"] = np.ascontiguousarray(g["a_Whh"].T).astype(bf16)
    m["wfcA"] = np.ascontiguousarray(g["a_Wfc"].T).astype(bf16)
    m["wm1"] = np.ascontiguousarray(g["m_W1"].T).astype(bf16)
    m["wm2"] = np.ascontiguousarray(g["m_W2"].T).astype(bf16)
    m["wuc1"] = np.ascontiguousarray(g["uc_W1"].T).astype(bf16)
    m["wuc2"] = np.ascontiguousarray(g["uc_W2"].T).astype(bf16)

    bias = np.zeros((128, NBIAS), f32)

    def putb(col, vec):
        vec = np.asarray(vec, f32).ravel()
        nt = (len(vec) + 127) // 128
        for k in range(nt):
            sl = vec[k * 128 : (k + 1) * 128]
            bias[: len(sl), col + k] = sl

    putb(BRZ_P, (g["p_bih"][: 2 * H] + g["p_bhh"][: 2 * H]) / 2)
    putb(BHN_P, g["p_bhh"][2 * H :])
    putb(BFC_P, g["p_bfc"])
    putb(BGP2, 4.0 * (g["gp_b2"] - g["gp_W2"].sum(1)))
    bias[:, BONEC] = 1.0
    putb(BNRZ_G, g["g_bhh"][: 2 * H] / 2)
    putb(BHN_G, g["g_bhh"][2 * H :])
    putb(BFC_G, g["g_bfc"])
    a_bih_adj = g["a_bih"] - g["a_Wih"].sum(1)
    putb(BNRZ_A, (a_bih_adj[: 2 * H] + g["a_bhh"][: 2 * H]) / 2)
    putb(BIN_A, a_bih_adj[2 * H :])
    putb(BHN_A, g["a_bhh"][2 * H :])
    putb(BFC_A, g["a_bfc"])
    putb(BM1, g["m_b1"] - g["m_W1"].sum(1))
    putb(BM2, g["m_b2"] - g["m_W2"].sum(1))
    putb(BUC1, g["uc_b1"] - g["uc_W1"].sum(1))
    bias[0, BUC2R : BUC2R + V] = (g["uc_b2"] - g["uc_W2"].sum(1)).astype(f32)
    bias[0, BONER : BONER + AC] = 1.0
    m["biases"] = bias
    return m


def kernel(**inputs) -> np.ndarray:
    nc = _build()
    in_maps = [_host_inputs(c, inputs) for c in range(NCORES)]
    res = run_bass_kernel_spmd(nc, in_maps, core_ids=list(range(NCORES)))
    total = np.float32(0.0)
    for r in res.results:
        total += np.float32(r["cost"].reshape(-1)[0])
    return np.array([total], np.float32)


if __name__ == "__main__":
    _build()
    print("build ok")
